# revision 1
# baseline (speedup 1.0000x reference)
"""Causal single-head attention (B=4, T=4096, C=1024, H=64) on 8 TRN2 NeuronCores.

Sharding: core = 2*b + p handles batch b and the 16 query/key row-blocks
(128 rows each) of parity p (block-cyclic over T for causal load balance).
Each core projects q/k/v for its own rows from a host-pretransposed x^T
slice, the core pair exchanges k^T/v^T per 512-column tile via AllGather,
and attention runs in the transposed orientation:
    S^T[s,t] = k^T.T @ q^T   (f32r matmuls, N=512 full rate)
    P^T = exp(S^T / 8)       (ACT, causality via 0/1 mask data per parity)
    out^T[h,t] = [v|1].T @ P^T  (row 64 accumulates softmax denominators)
then PE-transpose + normalize + DMA out.
"""
import numpy as np

import concourse.bacc as bacc
import concourse.bass as bass
import concourse.mybir as mybir
import concourse.tile as tile

dt = mybir.dt
F32R = dt.float32r
F32 = dt.float32

B, T, C, H = 4, 4096, 1024, 64
NBLK = T // 128            # 32 global blocks per batch
NLOC = NBLK // 2           # 16 blocks per core
NT = NLOC * 128            # 2048 query rows per core
NTT = NT // 512            # 4 t-tiles per core
N_CORES = 8
GROUPS = [[0, 1], [2, 3], [4, 5], [6, 7]]
SCALE = 1.0 / np.sqrt(H)

EXP = mybir.ActivationFunctionType.Exp


def _emit_body(nc, tc, aps, pools, rep):
    (xT_ap, wqk_ap, wv_ap, masks_ap, ident_ap, ones_ap, out_ap) = aps
    sb, ps, dr = pools

    # --- constants in SBUF ---
    wqk = sb.tile([128, 8 * 128], F32R, tag="wqk", name=f"wqk{rep}")
    wv = sb.tile([128, 8 * 64], F32R, tag="wv", name=f"wv{rep}")
    masks = sb.tile([128, 8 * 128], F32R, tag="masks", name=f"masks{rep}")
    identr = sb.tile([128, 128], F32R, tag="identr", name=f"identr{rep}")
    ident32 = sb.tile([128, 128], F32, tag="ident32", name=f"ident32{rep}")
    nc.sync.dma_start(wqk[:].rearrange("p (g h) -> p g h", g=8),
                      wqk_ap[:].rearrange("(g p) h -> p g h", p=128).bitcast(F32R))
    nc.sync.dma_start(wv[:].rearrange("p (g h) -> p g h", g=8),
                      wv_ap[:].rearrange("(g p) h -> p g h", p=128).bitcast(F32R))
    nc.sync.dma_start(masks[:], masks_ap[:].bitcast(F32R))
    nc.sync.dma_start(identr[:], ident_ap[:].bitcast(F32R))
    nc.sync.dma_start(ident32[:], ident_ap[:])

    # --- persistent activations ---
    qT_sb = sb.tile([64, NT], F32R, tag="qT", name=f"qT{rep}")
    kT_par = [sb.tile([64, NT], F32R, tag=f"kTp{j}", name=f"kTp{j}_{rep}") for j in (0, 1)]
    vT_par = [sb.tile([64, NT], F32R, tag=f"vTp{j}", name=f"vTp{j}_{rep}") for j in (0, 1)]
    vaug = [sb.tile([128, 65], F32R, tag=f"vaug{s}", name=f"vaug{s}_{rep}")
            for s in range(NBLK)]
    ones_sb = sb.tile([128, 1], F32R, tag="ones", name=f"ones{rep}")
    nc.sync.dma_start(ones_sb[:], ones_ap[:].bitcast(F32R))

    xT_3d = xT_ap[:].rearrange("(g p) n -> p g n", p=128)          # [128, 8, NT]

    # ---- stage A: projections + pair exchange + v_aug for one t-tile ----
    def emit_proj(tau):
        t0 = 512 * tau
        xt = sb.tile([128, 8 * 512], F32R, tag="xt", bufs=2, name=f"xt{rep}_{tau}")
        for c in range(8):  # chunked: matmuls start after the first 256KB
            nc.sync.dma_start(xt[:, 512*c:512*(c+1)],
                              xT_3d[:, c, t0:t0+512].bitcast(F32R))
        qkp = ps.tile([128, 512], F32, tag="qkp", name=f"qkp{rep}_{tau}")
        vp = ps.tile([64, 512], F32, tag="vp", name=f"vp{rep}_{tau}")
        for c in range(8):
            nc.tensor.matmul(qkp[:], wqk[:, 128*c:128*(c+1)], xt[:, 512*c:512*(c+1)],
                             start=(c == 0), stop=(c == 7))
        for c in range(8):
            nc.tensor.matmul(vp[:], wv[:, 64*c:64*(c+1)], xt[:, 512*c:512*(c+1)],
                             start=(c == 0), stop=(c == 7))
        kv = sb.tile([128, 512], F32R, tag="kv", bufs=2, name=f"kv{rep}_{tau}")
        nc.vector.tensor_copy(qT_sb[:, t0:t0+512], qkp[0:64, :])
        nc.vector.tensor_copy(kv[0:64, :], qkp[64:128, :])
        nc.vector.tensor_copy(kv[64:128, :], vp[:])

        ccin = dr.tile([128, 512], F32, tag="ccin", bufs=2, name=f"ccin{rep}_{tau}")
        ccout = dr.tile([2, 128, 512], F32, tag="ccout", bufs=2, name=f"ccout{rep}_{tau}")
        with tc.high_priority():
            nc.sync.dma_start(ccin[:], kv[:].bitcast(F32))
            if nc.num_devices > 1:
                nc.gpsimd.collective_compute(
                    "AllGather", mybir.AluOpType.bypass, replica_groups=GROUPS,
                    ins=[ccin[:]], outs=[ccout[:]],
                )
            else:  # single-core timing sim: stand-in DMAs with the same traffic
                nc.sync.dma_start(ccout[0], ccin[:])
                nc.sync.dma_start(ccout[1], ccin[:])
            for j in (0, 1):
                nc.sync.dma_start(kT_par[j][:, t0:t0+512], ccout[j, 0:64, :].bitcast(F32R))
                nc.sync.dma_start(vT_par[j][:, t0:t0+512], ccout[j, 64:128, :].bitcast(F32R))

        for s in range(8 * tau, 8 * tau + 8):
            tp = ps.tile([128, 64], F32R, tag="tr", name=f"trv{rep}_{s}")
            nc.tensor.transpose(tp[:], vT_par[s % 2][:, (s // 2)*128:(s // 2 + 1)*128],
                                identr[0:64, 0:64])
            nc.vector.tensor_copy(vaug[s][:, 0:64], tp[:])
            nc.vector.tensor_copy(vaug[s][:, 64:65], ones_sb[:])

    # ---- stage B: attention for one t-tile ----
    def emit_attn(tau):
        t0 = 512 * tau
        op = ps.tile([65, 512], F32, tag="outp", name=f"outp{rep}_{tau}")
        n_sig = 8 * tau + 8
        for m in range(n_sig // 2):
            s0, s1 = 2 * m, 2 * m + 1
            d0 = s0 - 8 * tau
            k = d0 // 2 if d0 >= 0 else 0
            off = 128 * k
            sp = ps.tile([128, 1024], F32, tag="sc", bufs=2, name=f"sc{rep}_{tau}_{m}")
            for idx, s in ((0, s0), (1, s1)):
                nc.tensor.matmul(
                    sp[:, 512*idx+off:512*(idx+1)],
                    kT_par[s % 2][:, (s // 2)*128:(s // 2 + 1)*128],
                    qT_sb[:, t0+off:t0+512],
                    start=True, stop=True)
            pt = sb.tile([128, 1024], F32R, tag="pt", bufs=3, name=f"pt{rep}_{tau}_{m}")
            sp_seg = sp[:].rearrange("p (s n) -> p s n", s=2)[:, :, off:512]
            pt_seg = pt[:].rearrange("p (s n) -> p s n", s=2)[:, :, off:512]
            nc.scalar.activation(pt_seg, sp_seg, EXP, scale=float(SCALE))
            if d0 >= 0:
                for idx, d in ((0, d0), (1, d0 + 1)):
                    seg = pt[:, 512*idx+off:512*idx+off+128]
                    nc.vector.tensor_mul(seg, seg, masks[:, 128*d:128*(d+1)])
            for idx, s in ((0, s0), (1, s1)):
                nc.tensor.matmul(
                    op[0:65, off:512], vaug[s][:], pt[:, 512*idx+off:512*(idx+1)],
                    start=(s == 0), stop=(s == n_sig - 1))

        # finalize: transpose back, normalize, one store per t-tile
        ob = sb.tile([65, 512], F32, tag="ob", bufs=2, name=f"ob{rep}_{tau}")
        nc.vector.tensor_copy(ob[:], op[:])
        ofin = sb.tile([128, 4 * 64], F32, tag="ofin", bufs=2, name=f"of{rep}_{tau}")
        for sub in range(4):
            tp2 = ps.tile([128, 65], F32, tag="tr", name=f"tr2{rep}_{tau}_{sub}")
            nc.tensor.transpose(tp2[:], ob[:, 128*sub:128*(sub+1)], ident32[0:65, 0:65])
            rc = sb.tile([128, 1], F32, tag="rc", bufs=2, name=f"rc{rep}_{tau}_{sub}")
            nc.vector.reciprocal(rc[:], tp2[:, 64:65])
            nc.vector.tensor_scalar_mul(ofin[:, 64*sub:64*(sub+1)], tp2[:, 0:64], rc[:])
        out_3d = out_ap[512*tau:512*(tau+1), :].rearrange("(s p) h -> p s h", p=128)
        nc.sync.dma_start(out_3d, ofin[:].rearrange("p (s h) -> p s h", s=4))

    # software pipeline: keep proj/exchange one t-tile ahead of attention
    for tau in range(NTT):
        emit_proj(tau)
        if tau >= 1:
            emit_attn(tau - 1)
    emit_attn(NTT - 1)


def build(reps=1, n_devices=N_CORES):
    nc = bacc.Bacc("TRN2", target_bir_lowering=False, debug=False,
                   num_devices=n_devices)
    xT_ap = nc.dram_tensor("xT", [C, NT], F32, kind="ExternalInput").ap()
    wqk_ap = nc.dram_tensor("wqk", [C, 128], F32, kind="ExternalInput").ap()
    wv_ap = nc.dram_tensor("wv", [C, 64], F32, kind="ExternalInput").ap()
    masks_ap = nc.dram_tensor("masks", [128, 8 * 128], F32, kind="ExternalInput").ap()
    ident_ap = nc.dram_tensor("ident", [128, 128], F32, kind="ExternalInput").ap()
    ones_ap = nc.dram_tensor("ones", [128, 1], F32, kind="ExternalInput").ap()
    out_ap = nc.dram_tensor("out", [NT, H], F32, kind="ExternalOutput").ap()
    aps = (xT_ap, wqk_ap, wv_ap, masks_ap, ident_ap, ones_ap, out_ap)

    with tile.TileContext(nc) as tc:
        with tc.tile_pool(name="sb", bufs=1) as sb, \
             tc.tile_pool(name="ps", bufs=1, space="PSUM") as ps, \
             tc.tile_pool(name="dr", bufs=1, space="DRAM") as dr:
            for rep in range(reps):
                _emit_body(nc, tc, aps, (sb, ps, dr), rep)
    nc.compile()
    return nc


def make_inputs(x, Wq, Wk, Wv):
    """Per-core input maps from full inputs."""
    x = np.asarray(x, dtype=np.float32)
    Wq, Wk, Wv = (np.asarray(w, dtype=np.float32) for w in (Wq, Wk, Wv))
    wqk = np.concatenate([Wq, Wk], axis=1)                      # [C, 128]
    tri = (np.arange(128)[:, None] <= np.arange(128)[None, :]).astype(np.float32)
    zeros = np.zeros((128, 128), np.float32)
    ones = np.ones((128, 128), np.float32)
    masks_even = np.concatenate([tri if d % 2 == 0 else zeros for d in range(8)], axis=1)
    masks_odd = np.concatenate([tri if d % 2 == 1 else ones for d in range(8)], axis=1)
    ident = np.eye(128, dtype=np.float32)
    ones_row = np.ones((128, 1), np.float32)

    in_maps = []
    for core in range(N_CORES):
        b, p = core // 2, core % 2
        xT = np.ascontiguousarray(
            x[b].T.reshape(C, NBLK, 128)[:, p::2, :].reshape(C, NT))
        in_maps.append({
            "xT": xT, "wqk": wqk, "wv": np.ascontiguousarray(Wv),
            "masks": masks_even if p == 0 else masks_odd,
            "ident": ident, "ones": ones_row,
        })
    return in_maps


def gather_output(results):
    """results: list per core of {"out": [NT, H]} → [B, T, H]."""
    out = np.empty((B, T, H), dtype=np.float32)
    for core in range(N_CORES):
        b, p = core // 2, core % 2
        o = results[core]["out"].reshape(NLOC, 128, H)
        out[b].reshape(NBLK, 128, H)[p::2] = o
    return out


# ---------------------------------------------------------------------------
# held PJRT runner (axon path) — inlined so kernel.py is self-contained
# ---------------------------------------------------------------------------

def make_runner(nc, n_cores):
    import jax
    from jax.sharding import Mesh, PartitionSpec
    from jax.experimental.shard_map import shard_map
    from concourse import bass2jax
    from concourse.bass2jax import _bass_exec_p, install_neuronx_cc_hook

    install_neuronx_cc_hook()
    partition_name = nc.partition_id_tensor.name if nc.partition_id_tensor else None

    in_names, out_names, out_avals, zero_shapes = [], [], [], []
    for alloc in nc.m.functions[0].allocations:
        if not isinstance(alloc, mybir.MemoryLocationSet):
            continue
        name = alloc.memorylocations[0].name
        if alloc.kind == "ExternalInput":
            if name != partition_name:
                in_names.append(name)
        elif alloc.kind == "ExternalOutput":
            out_names.append(name)
            shape = tuple(alloc.tensor_shape)
            dtype = mybir.dt.np(alloc.dtype)
            out_avals.append(jax.core.ShapedArray(shape, dtype))
            zero_shapes.append((shape, dtype))
    n_params, n_outs = len(in_names), len(out_avals)
    all_in_names = list(in_names) + list(out_names)
    if partition_name is not None:
        all_in_names.append(partition_name)
    donate = tuple(range(n_params, n_params + n_outs))

    def _body(*args):
        operands = list(args)
        if partition_name is not None:
            operands.append(bass2jax.partition_id_tensor())
        outs = _bass_exec_p.bind(
            *operands, out_avals=tuple(out_avals), in_names=tuple(all_in_names),
            out_names=tuple(out_names), lowering_input_output_aliases=(),
            sim_require_finite=True, sim_require_nnan=True, nc=nc)
        return tuple(outs)

    devices = jax.devices()[:n_cores]
    mesh = Mesh(np.asarray(devices), ("core",))
    sharded = jax.jit(
        shard_map(_body, mesh=mesh,
                  in_specs=(PartitionSpec("core"),) * (n_params + n_outs),
                  out_specs=(PartitionSpec("core"),) * n_outs, check_rep=False),
        donate_argnums=donate, keep_unused=True)
    make_zeros = jax.jit(lambda: tuple(
        jax.numpy.zeros((n_cores * s[0], *s[1:]), d) for (s, d) in zero_shapes))

    class Runner:
        def commit_inputs(self, in_maps):
            per_core = [[np.asarray(m[name]) for name in in_names] for m in in_maps]
            concat = [np.concatenate([per_core[c][i] for c in range(n_cores)], axis=0)
                      for i in range(n_params)]
            self._committed = [jax.device_put(a) for a in concat]
            jax.block_until_ready(self._committed)

        def run(self):
            outs = sharded(*self._committed, *make_zeros())
            jax.block_until_ready(outs)
            return outs

        def results(self, outs):
            res = [dict() for _ in range(n_cores)]
            for i, name in enumerate(out_names):
                per = np.split(np.asarray(outs[i]), n_cores, axis=0)
                for c in range(n_cores):
                    res[c][name] = per[c]
            return res

    return Runner()


_cache = {}


def get_runner(reps=1):
    if reps not in _cache:
        nc = build(reps)
        _cache[reps] = make_runner(nc, N_CORES)
    return _cache[reps]


def kernel(x, Wq, Wk, Wv):
    r = get_runner(1)
    r.commit_inputs(make_inputs(x, Wq, Wk, Wv))
    return gather_output(r.results(r.run()))



# revision 45
# speedup vs baseline: 1.1602x; 1.1602x over previous
"""Causal single-head attention (B=4, T=4096, C=1024, H=64) on 8 TRN2 NeuronCores.

Sharding: core = 2*b + p handles batch b and the 16 query/key row-blocks
(128 rows each) of parity p (block-cyclic over T for causal load balance).
The instruction stream is parity-agnostic (SPMD); causality parity is
carried by per-core 0/1 mask data (p=0: [tril, zeros], p=1: [ones, tril]).

All activations/weights in bf16 (x converted on host). Per t-tile of 512
local tokens the core projects q^T,k^T (transposed, H-major) and v
(token-major) from one prefetched x^T slice, the core pair exchanges
k^T/v per tile via one AllGather (bf16), and attention runs per key
block s with near-exact causal spans:
    S^T[s,*] = kT_s.T @ qT[span]      (bf16 matmul, f32 psum)
    P^T = 2^(S^T)                     (scale 1/(8*ln2... log2e/8) folded
                                       into Wq; split ACT exp(ln2*x) and
                                       DVE pow(2,x) by greedy balance)
    mask[s%2] on first 128 cols       (DVE bf16 mul)
    out[q,65] += P^T_block.T @ [v|1]  (weights-stationary, 65-cycle matmuls;
                                       col 64 accumulates softmax denom)
then reciprocal-normalize (DVE) and one store per t-tile (Pool swdge).
"""
import numpy as np

import concourse.bacc as bacc
import concourse.bass as bass
import concourse.mybir as mybir
import concourse.tile as tile

dt = mybir.dt
BF16 = dt.bfloat16
F32 = dt.float32

B, T, C, H = 4, 4096, 1024, 64
NBLK = T // 128            # 32 global blocks per batch
NLOC = NBLK // 2           # 16 blocks per core
NT = NLOC * 128            # 2048 query rows per core
NTT = NT // 512            # 4 t-tiles per core
N_CORES = 8
GROUPS = [[0, 1], [2, 3], [4, 5], [6, 7]]
LOG2E = float(np.log2(np.e))
LN2 = float(np.log(2.0))

EXP = mybir.ActivationFunctionType.Exp
ALU = mybir.AluOpType

# exp engine balance: effective ns per psum column + per-op overhead,
# plus an initial credit for each engine's non-exp work. GPSIMD cannot
# read PSUM on hardware, so only ACT and DVE run the exp.
EXP_COST = [  # (ns_per_col, ns_per_op, initial_credit)
    (1 / 1.2, 185.0, 0.0),        # ACT
    (1 / 0.96, 125.0, 16000.0),   # DVE: copies + half the masks + normalize
]


def _pairs(tau):
    """Key blocks (2m, 2m+1) share one span: (m, col0 within the 512-tile).
    Parity-agnostic superset structure (p=1 exact; p=0 cores zero the s-odd
    diagonal via mask data)."""
    return [(m, 128 * max(0, m - 4 * tau)) for m in range(4 * tau + 4)]


def _emit_body(nc, tc, aps, pools, rep):
    (xT_ap, wqk_ap, wv_ap, masks_ap, out_ap) = aps
    sb, ps, dr = pools

    # --- constants ---
    wqk = sb.tile([128, 8 * 128], BF16, tag="wqk", name=f"wqk{rep}")
    wv = sb.tile([128, 8 * 64], BF16, tag="wv", name=f"wv{rep}")
    masks = sb.tile([128, 2 * 128], BF16, tag="masks", name=f"masks{rep}")
    two = sb.tile([128, 1], BF16, tag="two", name=f"two{rep}")
    nc.sync.dma_start(wqk[:], wqk_ap[:])
    nc.sync.dma_start(wv[:], wv_ap[:])
    nc.sync.dma_start(masks[:], masks_ap[:])
    nc.vector.memset(two[:], 2.0)

    # --- persistent activations ---
    # kv_sb: packed exchange mirror, one 772-col region per (tau, j):
    #   cols [0:260]   = [v|1] per key block q: [65q : 65q+65] (token-major)
    #   cols [260:772] = kT, rows 0:64 (H-major, 128 cols per block q);
    #   rows 64:128 of that range ride along unused (keeps matmul operands
    #   at base partition 0)
    kv_sb = sb.tile([128, 8 * 772], BF16, tag="kv", name=f"kv{rep}")

    def kv_base(s):
        return 772 * (2 * (s // 8) + (s % 2))

    def pv_rhs(s):
        q = (s % 8) // 2
        return kv_sb[:, kv_base(s)+65*q:kv_base(s)+65*q+65]

    def s_lhsT(s):
        q = (s % 8) // 2
        c = kv_base(s) + 260 + 128 * q
        return kv_sb[0:64, c:c+128]

    # x tiles: loaded in halves, staggered (xt0 upfront, xt(tau+1) after
    # proj(tau)'s exchange DMAs so the exchange wins the DMA-engine queue)
    xT_3d = xT_ap[:].rearrange("(g p) n -> p g n", p=128)          # [128,8,NT]
    xts = [sb.tile([128, 8 * 512], BF16, tag=f"xt{tau}", name=f"xt{rep}_{tau}")
           for tau in range(NTT)]

    def load_xt(tau):
        for h in range(4):
            nc.sync.dma_start(
                xts[tau][:].rearrange("p (g n) -> p g n", g=8)[:, 2*h:2*h+2, :],
                xT_3d[:, 2*h:2*h+2, 512*tau:512*tau+512])

    load_xt(0)

    qk_own = [sb.tile([128, 512], BF16, tag=f"qk{tau}", name=f"qk{rep}_{tau}")
              for tau in range(NTT)]

    # warm up the PE p-state ramp while weights/x stream in; scratch input
    # comes from a memset so the warmup has no DMA dependency
    wsrc = sb.tile([128, 512], BF16, tag="wsrc", name=f"wsrc{rep}")
    nc.vector.memset(wsrc[:], 0.25)
    warm = ps.tile([128, 1024], F32, tag="sc", bufs=2, name=f"warm{rep}")
    for c in range(8):
        nc.tensor.matmul(warm[:, 0:512], wsrc[:, 0:128], wsrc[:],
                         start=(c == 0), stop=(c == 7))

    # ---- stage A: projections + pair exchange for one t-tile ----
    def emit_proj(tau):
        t0 = 512 * tau
        xt = xts[tau]
        qkp = ps.tile([128, 512], F32, tag="qkp", bufs=1, name=f"qkp{rep}_{tau}")
        vp = ps.tile([128, 256], F32, tag="vp", bufs=1,
                     name=f"vp{rep}_{tau}")
        for c in range(8):
            nc.tensor.matmul(qkp[:], wqk[:, 128*c:128*(c+1)], xt[:, 512*c:512*(c+1)],
                             start=(c == 0), stop=(c == 7))
        for tb in range(4):
            for c in range(8):
                nc.tensor.matmul(vp[:, 64*tb:64*(tb+1)],
                                 xt[:, 512*c+128*tb:512*c+128*(tb+1)],
                                 wv[:, 64*c:64*(c+1)],
                                 start=(c == 0), stop=(c == 7))
        # psum -> sbuf bf16; v_own carries the ones column per block
        v_own = sb.tile([128, 260], BF16, tag="vown", bufs=2, name=f"vo{rep}_{tau}")
        nc.vector.tensor_copy(qk_own[tau][:], qkp[:])
        v_own3 = v_own[:].rearrange("p (q c) -> p q c", c=65)
        nc.vector.tensor_copy(v_own3[:, :, 0:64],
                              vp[:].rearrange("p (q c) -> p q c", q=4))
        nc.vector.memset(v_own3[:, :, 64], 1.0)

        # pair exchange: ckv = [ v|1 (128,260) | kT (64,512) ]
        ckv = dr.tile([128, 772], BF16, tag="ckv", bufs=2, name=f"ckv{rep}_{tau}")
        ccout = dr.tile([2, 128, 772], BF16, tag="ccout", bufs=2,
                        name=f"ccout{rep}_{tau}")
        with tc.high_priority():
            nc.gpsimd.dma_start(ckv[:, 0:260], v_own[:])
            nc.gpsimd.dma_start(ckv[0:64, 260:772], qk_own[tau][64:128, :])
            if nc.num_devices > 1:
                nc.gpsimd.collective_compute(
                    "AllGather", ALU.bypass, replica_groups=GROUPS,
                    ins=[ckv[:]], outs=[ccout[:]],
                )
            else:  # single-core timing sim: stand-in DMAs, same traffic
                nc.sync.dma_start(ccout[0], ckv[:])
                nc.sync.dma_start(ccout[1], ckv[:])
            # both pair slices, incl. own (parity-agnostic), contiguous
            for j in (0, 1):
                base = 772 * (2 * tau + j)
                nc.sync.dma_start(kv_sb[:, base:base+772], ccout[j])
        if tau + 1 < NTT:
            load_xt(tau + 1)

    # ---- stage B: attention, ordered by exchange phase ----
    # After exchange(e) lands, blocks s in [8e, 8e+8) are computable for every
    # tile tp >= e; exchange(e+1) hides behind that whole phase. Key blocks
    # are processed in (2m, 2m+1) pairs sharing one span so exp runs as one
    # ACT op per pair half. A psum bank holds only ONE open accumulation
    # group at a time (hardware: a start=True wipes the bank's open context),
    # so PV accumulates per (phase, tile) into a psum partial with strictly
    # sequential per-qb groups, then merges into SBUF running accumulators.
    def emit_attn_all():
        pairs = []                      # (tp, m, c0) in phase order
        for e in range(NTT):
            for tp in range(e, NTT):
                for m in range(4 * e, 4 * e + 4):
                    pairs.append((tp, m, 128 * max(0, m - 4 * tp)))
        n = len(pairs)
        accs = [sb.tile([128, 4 * 65], F32, tag=f"acc{tp}", name=f"acc{rep}_{tp}")
                for tp in range(NTT)]
        ofins = [sb.tile([128, 4 * 64], F32, tag=f"ofin{tp}", name=f"of{rep}_{tp}")
                 for tp in range(NTT)]
        sps, pts = [None] * n, [None] * n

        def emit_S(k):
            tp, m, c0 = pairs[k]
            w = 512 - c0
            sp_t = ps.tile([128, 1024], F32, tag="sc", bufs=2, name=f"sc{rep}_{k}")
            for i, s in ((0, 2 * m), (1, 2 * m + 1)):
                nc.tensor.matmul(sp_t[:, 512*i:512*i+w], s_lhsT(s),
                                 qk_own[tp][0:64, c0:512], start=True, stop=True)
            sps[k] = sp_t

        def emit_exp_pair(k):
            tp, m, c0 = pairs[k]
            w = 512 - c0
            pt_t = sb.tile([128, 1024], BF16, tag="pt", bufs=6, name=f"pt{rep}_{k}")
            for i in (0, 1):
                nc.scalar.activation(pt_t[:, 512*i:512*i+w],
                                     sps[k][:, 512*i:512*i+w], EXP, scale=LN2)
            pts[k] = pt_t

        def emit_mask(k):
            tp, m, c0 = pairs[k]
            if m < 4 * tp:
                return  # off-diagonal pair: fully kept, no mask
            # both parities' diagonal query block (first 128 cols of each half)
            for i in (0, 1):
                nc.vector.tensor_mul(pts[k][:, 512*i:512*i+128],
                                     pts[k][:, 512*i:512*i+128],
                                     masks[:, 128*i:128*(i+1)])

        def emit_pv_phase(e, tp, ks):
            """PV for tile tp over this phase's four pairs `ks`, one complete
            psum group per qb (never two open groups in the bank), then merge
            into the SBUF accumulator (and normalize/store on the diagonal
            phase e == tp)."""
            php = ps.tile([128, 4 * 65], F32, tag="php", bufs=2,
                          name=f"php{rep}_{e}_{tp}")
            for qb in range(4):
                go = 2 * (4 * tp + qb) + 1
                mms = []
                for k in ks:
                    _, m, c0 = pairs[k]
                    for i, s in ((0, 2 * m), (1, 2 * m + 1)):
                        if s <= go:
                            mms.append((k, i, s, c0))
                for j, (k, i, s, c0) in enumerate(mms):
                    nc.tensor.matmul(
                        php[:, 65*qb:65*(qb+1)],
                        pts[k][:, 512*i+128*qb-c0:512*i+128*(qb+1)-c0],
                        pv_rhs(s),
                        start=(j == 0), stop=(j == len(mms) - 1))
            if e == 0:
                nc.vector.tensor_copy(accs[tp][:], php[:])
            else:
                nc.vector.scalar_tensor_tensor(accs[tp][:], php[:], 0.0,
                                               accs[tp][:], ALU.bypass, ALU.add)
            if e == tp:  # diagonal phase: normalize + store
                a3 = accs[tp][:].rearrange("p (q c) -> p q c", c=65)
                rc = sb.tile([128, 4], F32, tag="rc", bufs=2,
                             name=f"rc{rep}_{tp}")
                nc.vector.reciprocal(rc[:], a3[:, :, 64])
                for qb in range(4):
                    nc.vector.tensor_scalar_mul(ofins[tp][:, 64*qb:64*(qb+1)],
                                                a3[:, qb, 0:64], rc[:, qb:qb+1])
                if tp == NTT - 1:
                    nc.sync.dma_start(out_ap[128*tp:128*(tp+1), :], ofins[tp][:])
                else:
                    nc.gpsimd.dma_start(out_ap[128*tp:128*(tp+1), :], ofins[tp][:])

        # ACT (the sole exp engine) is the attention rate-limiter: keep its
        # queue pure exp and never input-starved; S runs two pairs ahead,
        # masks trail one pair, PV fires once a tile-phase's pairs are done.
        for k in range(min(2, n)):
            emit_S(k)
        for k in range(n + 1):
            if k < n:
                emit_exp_pair(k)
            if k >= 1:
                emit_mask(k - 1)
            if k + 2 < n:
                emit_S(k + 2)
            if k >= 1 and k % 4 == 0:
                kk = k - 4
                tp, m, _ = pairs[kk]
                emit_pv_phase(m // 4, tp, [kk, kk + 1, kk + 2, kk + 3])

    # all projections first (exchange latency hides behind them), then the
    # phase-ordered attention stream; filler matmuls keep the PE p-state hot
    # while the first exchange completes
    for tau in range(NTT):
        emit_proj(tau)
    if DEBUG_DUMP:
        kvd = nc.dram_tensor("kvdump", [128, 8 * 772], BF16,
                             kind="ExternalOutput").ap()
        qkd = nc.dram_tensor("qkdump", [128, 4 * 512], BF16,
                             kind="ExternalOutput").ap()
        nc.sync.dma_start(kvd[:], kv_sb[:])
        for t in range(NTT):
            nc.sync.dma_start(qkd[:, 512*t:512*(t+1)], qk_own[t][:])
    warm2 = ps.tile([128, 1024], F32, tag="sc", bufs=2, name=f"warm2{rep}")
    for c in range(8):
        nc.tensor.matmul(warm2[:, 0:512], wsrc[:, 0:128], wsrc[:],
                         start=(c == 0), stop=(c == 7))
    emit_attn_all()


DEBUG_DUMP = False


def build(reps=1, n_devices=N_CORES):
    nc = bacc.Bacc("TRN2", target_bir_lowering=False, debug=False,
                   num_devices=n_devices)
    xT_ap = nc.dram_tensor("xT", [C, NT], BF16, kind="ExternalInput").ap()
    wqk_ap = nc.dram_tensor("wqk", [128, 8 * 128], BF16,
                            kind="ExternalInput").ap()
    wv_ap = nc.dram_tensor("wv", [128, 8 * 64], BF16, kind="ExternalInput").ap()
    masks_ap = nc.dram_tensor("masks", [128, 2 * 128], BF16,
                              kind="ExternalInput").ap()
    # out rows: (tau, t) pairs; cols: (qb, h) -> local token = tau*512+qb*128+t
    out_ap = nc.dram_tensor("out", [NTT * 128, 4 * H], F32,
                            kind="ExternalOutput").ap()
    aps = (xT_ap, wqk_ap, wv_ap, masks_ap, out_ap)

    with tile.TileContext(nc) as tc:
        with tc.tile_pool(name="sb", bufs=1) as sb, \
             tc.tile_pool(name="ps", bufs=1, space="PSUM") as ps, \
             tc.tile_pool(name="dr", bufs=1, space="DRAM") as dr:
            for rep in range(reps):
                _emit_body(nc, tc, aps, (sb, ps, dr), rep)
    nc.compile()
    return nc


def make_inputs(x, Wq, Wk, Wv):
    """Per-core input maps from full inputs."""
    x = np.asarray(x, dtype=np.float32)
    Wq, Wk, Wv = (np.asarray(w, dtype=np.float32) for w in (Wq, Wk, Wv))
    # fold softmax scale and base-2 conversion into Wq: S' = log2(e)/sqrt(H)*qk
    wqk = np.concatenate([Wq * (LOG2E / np.sqrt(H)), Wk], axis=1)
    tril = (np.arange(128)[:, None] <= np.arange(128)[None, :]).astype(np.float32)
    zeros = np.zeros((128, 128), np.float32)
    ones = np.ones((128, 128), np.float32)
    masks_even = np.concatenate([tril, zeros], axis=1)   # p=0: diag at even s
    masks_odd = np.concatenate([ones, tril], axis=1)     # p=1: diag at odd s

    ml = mybir.dt.np(BF16)
    # pre-arrange weights into the SBUF chunk layout [128, chunks*cols]
    wqk16 = np.ascontiguousarray(
        wqk.reshape(8, 128, 128).transpose(1, 0, 2).reshape(128, 1024)).astype(ml)
    wv16 = np.ascontiguousarray(
        Wv.reshape(8, 128, 64).transpose(1, 0, 2).reshape(128, 512)).astype(ml)

    in_maps = []
    for core in range(N_CORES):
        b, p = core // 2, core % 2
        xT = np.ascontiguousarray(
            x[b].T.reshape(C, NBLK, 128)[:, p::2, :].reshape(C, NT)).astype(ml)
        in_maps.append({
            "xT": xT, "wqk": wqk16, "wv": wv16,
            "masks": (masks_even if p == 0 else masks_odd).astype(ml),
        })
    return in_maps


def gather_output(results):
    """results: list per core of {"out": [512, 256]} -> [B, T, H]."""
    out = np.empty((B, T, H), dtype=np.float32)
    for core in range(N_CORES):
        b, p = core // 2, core % 2
        o = results[core]["out"].reshape(NTT, 128, 4, H)
        o = o.transpose(0, 2, 1, 3).reshape(NLOC, 128, H)
        out[b].reshape(NBLK, 128, H)[p::2] = o
    return out


# ---------------------------------------------------------------------------
# held PJRT runner (axon path) — inlined so kernel.py is self-contained
# ---------------------------------------------------------------------------

def make_runner(nc, n_cores):
    import jax
    from jax.sharding import Mesh, PartitionSpec
    from jax.experimental.shard_map import shard_map
    from concourse import bass2jax
    from concourse.bass2jax import _bass_exec_p, install_neuronx_cc_hook

    install_neuronx_cc_hook()
    partition_name = nc.partition_id_tensor.name if nc.partition_id_tensor else None

    in_names, out_names, out_avals, zero_shapes = [], [], [], []
    for alloc in nc.m.functions[0].allocations:
        if not isinstance(alloc, mybir.MemoryLocationSet):
            continue
        name = alloc.memorylocations[0].name
        if alloc.kind == "ExternalInput":
            if name != partition_name:
                in_names.append(name)
        elif alloc.kind == "ExternalOutput":
            out_names.append(name)
            shape = tuple(alloc.tensor_shape)
            dtype = mybir.dt.np(alloc.dtype)
            out_avals.append(jax.core.ShapedArray(shape, dtype))
            zero_shapes.append((shape, dtype))
    n_params, n_outs = len(in_names), len(out_avals)
    all_in_names = list(in_names) + list(out_names)
    if partition_name is not None:
        all_in_names.append(partition_name)
    donate = tuple(range(n_params, n_params + n_outs))

    def _body(*args):
        operands = list(args)
        if partition_name is not None:
            operands.append(bass2jax.partition_id_tensor())
        outs = _bass_exec_p.bind(
            *operands, out_avals=tuple(out_avals), in_names=tuple(all_in_names),
            out_names=tuple(out_names), lowering_input_output_aliases=(),
            sim_require_finite=True, sim_require_nnan=True, nc=nc)
        return tuple(outs)

    devices = jax.devices()[:n_cores]
    mesh = Mesh(np.asarray(devices), ("core",))
    sharded = jax.jit(
        shard_map(_body, mesh=mesh,
                  in_specs=(PartitionSpec("core"),) * (n_params + n_outs),
                  out_specs=(PartitionSpec("core"),) * n_outs, check_rep=False),
        donate_argnums=donate, keep_unused=True)
    make_zeros = jax.jit(lambda: tuple(
        jax.numpy.zeros((n_cores * s[0], *s[1:]), d) for (s, d) in zero_shapes))

    class Runner:
        def commit_inputs(self, in_maps):
            per_core = [[np.asarray(m[name]) for name in in_names] for m in in_maps]
            concat = [np.concatenate([per_core[c][i] for c in range(n_cores)], axis=0)
                      for i in range(n_params)]
            self._committed = [jax.device_put(a) for a in concat]
            jax.block_until_ready(self._committed)

        def run(self):
            outs = sharded(*self._committed, *make_zeros())
            jax.block_until_ready(outs)
            return outs

        def results(self, outs):
            res = [dict() for _ in range(n_cores)]
            for i, name in enumerate(out_names):
                per = np.split(np.asarray(outs[i]), n_cores, axis=0)
                for c in range(n_cores):
                    res[c][name] = per[c]
            return res

    return Runner()


_cache = {}


def get_runner(reps=1):
    if reps not in _cache:
        nc = build(reps)
        _cache[reps] = make_runner(nc, N_CORES)
    return _cache[reps]


def kernel(x, Wq, Wk, Wv):
    r = get_runner(1)
    r.commit_inputs(make_inputs(x, Wq, Wk, Wv))
    return gather_output(r.results(r.run()))


# revision 50
# speedup vs baseline: 1.3660x; 1.1774x over previous
"""Causal single-head attention (B=4, T=4096, C=1024, H=64) on 8 TRN2 NeuronCores.

Sharding: core = 2*b + p handles batch b and the 16 query/key row-blocks
(128 rows each) of parity p (block-cyclic over T for causal load balance).
The instruction stream is parity-agnostic (SPMD); causality parity is
carried by per-core 0/1 mask data (p=0: [tril, zeros], p=1: [ones, tril]).

All activations/weights in bf16 (x converted on host). Per t-tile of 512
local tokens the core projects q^T,k^T (transposed, H-major) and v
(token-major) from one prefetched x^T slice, the core pair exchanges
k^T/v per tile via one AllGather (bf16), and attention runs per key
block s with near-exact causal spans:
    S^T[s,*] = kT_s.T @ qT[span]      (bf16 matmul, f32 psum)
    P^T = 2^(S^T)                     (scale 1/(8*ln2... log2e/8) folded
                                       into Wq; split ACT exp(ln2*x) and
                                       DVE pow(2,x) by greedy balance)
    mask[s%2] on first 128 cols       (DVE bf16 mul)
    out[q,65] += P^T_block.T @ [v|1]  (weights-stationary, 65-cycle matmuls;
                                       col 64 accumulates softmax denom)
then reciprocal-normalize (DVE) and one store per t-tile (Pool swdge).
"""
import numpy as np

import concourse.bacc as bacc
import concourse.bass as bass
import concourse.mybir as mybir
import concourse.tile as tile

dt = mybir.dt
BF16 = dt.bfloat16
F32 = dt.float32

B, T, C, H = 4, 4096, 1024, 64
NBLK = T // 128            # 32 global blocks per batch
NLOC = NBLK // 2           # 16 blocks per core
NT = NLOC * 128            # 2048 query rows per core
NTT = NT // 512            # 4 t-tiles per core
N_CORES = 8
GROUPS = [[0, 1], [2, 3], [4, 5], [6, 7]]
LOG2E = float(np.log2(np.e))
LN2 = float(np.log(2.0))

EXP = mybir.ActivationFunctionType.Exp
ALU = mybir.AluOpType

# exp engine balance: effective ns per psum column + per-op overhead,
# plus an initial credit for each engine's non-exp work. GPSIMD cannot
# read PSUM on hardware, so only ACT and DVE run the exp.
EXP_COST = [  # (ns_per_col, ns_per_op, initial_credit)
    (1 / 1.2, 185.0, 0.0),        # ACT
    (1 / 0.96, 125.0, 16000.0),   # DVE: copies + half the masks + normalize
]


def _pairs(tau):
    """Key blocks (2m, 2m+1) share one span: (m, col0 within the 512-tile).
    Parity-agnostic superset structure (p=1 exact; p=0 cores zero the s-odd
    diagonal via mask data)."""
    return [(m, 128 * max(0, m - 4 * tau)) for m in range(4 * tau + 4)]


def _emit_body(nc, tc, aps, pools, rep):
    (xT_ap, wqk_ap, wv_ap, masks_ap, out_ap) = aps
    sb, ps, dr = pools

    # --- constants ---
    wqk = sb.tile([128, 8 * 128], BF16, tag="wqk", name=f"wqk{rep}")
    wv = sb.tile([128, 8 * 64], BF16, tag="wv", name=f"wv{rep}")
    masks = sb.tile([128, 2 * 128], BF16, tag="masks", name=f"masks{rep}")
    two = sb.tile([128, 1], BF16, tag="two", name=f"two{rep}")
    nc.sync.dma_start(wqk[:], wqk_ap[:])
    nc.sync.dma_start(wv[:], wv_ap[:])
    nc.sync.dma_start(masks[:], masks_ap[:])
    nc.vector.memset(two[:], 2.0)

    # --- persistent activations ---
    # kv_sb: packed exchange mirror, one 772-col region per (tau, j):
    #   cols [0:260]   = [v|1] per key block q: [65q : 65q+65] (token-major)
    #   cols [260:772] = kT, rows 0:64 (H-major, 128 cols per block q);
    #   rows 64:128 of that range ride along unused (keeps matmul operands
    #   at base partition 0)
    kv_sb = sb.tile([128, 8 * 772], BF16, tag="kv", name=f"kv{rep}")

    def kv_base(s):
        return 772 * (2 * (s // 8) + (s % 2))

    def pv_rhs(s):
        q = (s % 8) // 2
        return kv_sb[:, kv_base(s)+65*q:kv_base(s)+65*q+65]

    def s_lhsT(s):
        q = (s % 8) // 2
        c = kv_base(s) + 260 + 128 * q
        return kv_sb[0:64, c:c+128]

    # x tiles: loaded in halves, staggered (xt0 upfront, xt(tau+1) after
    # proj(tau)'s exchange DMAs so the exchange wins the DMA-engine queue)
    xT_3d = xT_ap[:].rearrange("(g p) n -> p g n", p=128)          # [128,8,NT]
    xts = [sb.tile([128, 8 * 512], BF16, tag=f"xt{tau}", name=f"xt{rep}_{tau}")
           for tau in range(NTT)]

    def load_xt(tau):
        for h in range(4):
            nc.sync.dma_start(
                xts[tau][:].rearrange("p (g n) -> p g n", g=8)[:, 2*h:2*h+2, :],
                xT_3d[:, 2*h:2*h+2, 512*tau:512*tau+512])

    load_xt(0)

    qk_own = [sb.tile([128, 512], BF16, tag=f"qk{tau}", name=f"qk{rep}_{tau}")
              for tau in range(NTT)]

    # warm up the PE p-state ramp while weights/x stream in; scratch input
    # comes from a memset so the warmup has no DMA dependency
    wsrc = sb.tile([128, 512], BF16, tag="wsrc", name=f"wsrc{rep}")
    nc.vector.memset(wsrc[:], 0.25)
    warm = ps.tile([128, 1024], F32, tag="sc", bufs=2, name=f"warm{rep}")
    for c in range(8):
        nc.tensor.matmul(warm[:, 0:512], wsrc[:, 0:128], wsrc[:],
                         start=(c == 0), stop=(c == 7))

    # ---- stage A: projections + pair exchange for one t-tile ----
    # compute (PE matmuls + psum->sbuf copies + Pool stages) is emitted per
    # tau; the SP-queue exchange DMAs (stand-ins/collective readbacks) are
    # emitted separately so x-tile loads win the DMA-engine queue early on
    def emit_proj(tau):
        xt = xts[tau]
        qkp = ps.tile([128, 512], F32, tag="qkp", bufs=1, name=f"qkp{rep}_{tau}")
        vp = ps.tile([128, 256], F32, tag="vp", bufs=1,
                     name=f"vp{rep}_{tau}")
        for c in range(8):
            nc.tensor.matmul(qkp[:], wqk[:, 128*c:128*(c+1)], xt[:, 512*c:512*(c+1)],
                             start=(c == 0), stop=(c == 7))
        for tb in range(4):
            for c in range(8):
                nc.tensor.matmul(vp[:, 64*tb:64*(tb+1)],
                                 xt[:, 512*c+128*tb:512*c+128*(tb+1)],
                                 wv[:, 64*c:64*(c+1)],
                                 start=(c == 0), stop=(c == 7))
        # psum -> sbuf bf16; v_own carries the ones column per block
        v_own = sb.tile([128, 260], BF16, tag="vown", bufs=2, name=f"vo{rep}_{tau}")
        nc.vector.tensor_copy(qk_own[tau][:], qkp[:])
        v_own3 = v_own[:].rearrange("p (q c) -> p q c", c=65)
        nc.vector.tensor_copy(v_own3[:, :, 0:64],
                              vp[:].rearrange("p (q c) -> p q c", q=4))
        nc.vector.memset(v_own3[:, :, 64], 1.0)

        if tau == 0:
            # first exchange gates attention: k ships separately (S needs
            # only k), v follows while the first exps run
            ck = dr.tile([64, 512], BF16, tag="ck", name=f"ck{rep}")
            cv = dr.tile([128, 260], BF16, tag="cv", name=f"cv{rep}")
            nc.gpsimd.dma_start(ck[:], qk_own[0][64:128, :])
            nc.gpsimd.dma_start(cv[:], v_own[:])
            return (ck, cv)
        ckv = dr.tile([128, 772], BF16, tag="ckv", bufs=2, name=f"ckv{rep}_{tau}")
        nc.gpsimd.dma_start(ckv[:, 0:260], v_own[:])
        nc.gpsimd.dma_start(ckv[0:64, 260:772], qk_own[tau][64:128, :])
        return ckv

    def emit_exchange0_k(ck):
        ck_o = dr.tile([2, 64, 512], BF16, tag="cko", name=f"cko{rep}")
        if nc.num_devices > 1:
            nc.gpsimd.collective_compute(
                "AllGather", ALU.bypass, replica_groups=GROUPS,
                ins=[ck[:]], outs=[ck_o[:]])
        else:  # single-core timing sim: stand-ins, same traffic
            nc.sync.dma_start(ck_o[0], ck[:])
            nc.sync.dma_start(ck_o[1], ck[:])
        for j in (0, 1):
            nc.sync.dma_start(kv_sb[0:64, 772*j+260:772*j+772], ck_o[j])

    def emit_exchange0_v(cv):
        cv_o = dr.tile([2, 128, 260], BF16, tag="cvo", name=f"cvo{rep}")
        if nc.num_devices > 1:
            nc.gpsimd.collective_compute(
                "AllGather", ALU.bypass, replica_groups=GROUPS,
                ins=[cv[:]], outs=[cv_o[:]])
        else:
            nc.sync.dma_start(cv_o[0], cv[:])
            nc.sync.dma_start(cv_o[1], cv[:])
        for j in (0, 1):
            nc.sync.dma_start(kv_sb[:, 772*j:772*j+260], cv_o[j])

    def emit_exchange(tau, ckv):
        ccout = dr.tile([2, 128, 772], BF16, tag="ccout", bufs=2,
                        name=f"ccout{rep}_{tau}")
        if nc.num_devices > 1:
            nc.gpsimd.collective_compute(
                "AllGather", ALU.bypass, replica_groups=GROUPS,
                ins=[ckv[:]], outs=[ccout[:]],
            )
        else:  # single-core timing sim: stand-in DMAs, same traffic
            nc.sync.dma_start(ccout[0], ckv[:])
            nc.sync.dma_start(ccout[1], ckv[:])
        # both pair slices, incl. own (parity-agnostic), contiguous
        for j in (0, 1):
            base = 772 * (2 * tau + j)
            nc.sync.dma_start(kv_sb[:, base:base+772], ccout[j])

    # ---- stage B: attention, ordered by exchange phase ----
    # After exchange(e) lands, blocks s in [8e, 8e+8) are computable for every
    # tile tp >= e; exchange(e+1) hides behind that whole phase. Key blocks
    # are processed in (2m, 2m+1) pairs sharing one span so exp runs as one
    # ACT op per pair half. A psum bank holds only ONE open accumulation
    # group at a time (hardware: a start=True wipes the bank's open context),
    # so PV accumulates per (phase, tile) into a psum partial with strictly
    # sequential per-qb groups, then merges into SBUF running accumulators.
    def emit_attn_all():
        pairs = []                      # (tp, m, c0) in phase order
        for e in range(NTT):
            for tp in range(e, NTT):
                for m in range(4 * e, 4 * e + 4):
                    pairs.append((tp, m, 128 * max(0, m - 4 * tp)))
        n = len(pairs)
        accs = [sb.tile([128, 4 * 65], F32, tag=f"acc{tp}", name=f"acc{rep}_{tp}")
                for tp in range(NTT)]
        ofins = [sb.tile([128, 4 * 64], F32, tag=f"ofin{tp}", name=f"of{rep}_{tp}")
                 for tp in range(NTT)]
        sps, pts = [None] * n, [None] * n

        def emit_S(k):
            tp, m, c0 = pairs[k]
            w = 512 - c0
            sp_t = ps.tile([128, 1024], F32, tag="sc", bufs=2, name=f"sc{rep}_{k}")
            for i, s in ((0, 2 * m), (1, 2 * m + 1)):
                nc.tensor.matmul(sp_t[:, 512*i:512*i+w], s_lhsT(s),
                                 qk_own[tp][0:64, c0:512], start=True, stop=True)
            sps[k] = sp_t

        def emit_exp_pair(k):
            tp, m, c0 = pairs[k]
            w = 512 - c0
            pt_t = sb.tile([128, 1024], BF16, tag="pt", bufs=6, name=f"pt{rep}_{k}")
            sp3 = sps[k][:].rearrange("p (two c) -> p two c", two=2)[:, :, 0:w]
            pt3 = pt_t[:].rearrange("p (two c) -> p two c", two=2)[:, :, 0:w]
            nc.scalar.activation(pt3, sp3, EXP, scale=LN2)
            pts[k] = pt_t

        def emit_mask(k):
            tp, m, c0 = pairs[k]
            if m < 4 * tp:
                return  # off-diagonal pair: fully kept, no mask
            # both parities' diagonal query block (first 128 cols of each half)
            for i in (0, 1):
                nc.vector.tensor_mul(pts[k][:, 512*i:512*i+128],
                                     pts[k][:, 512*i:512*i+128],
                                     masks[:, 128*i:128*(i+1)])

        def emit_pv_phase(e, tp, ks):
            """PV for tile tp over this phase's four pairs `ks`, one complete
            psum group per qb (never two open groups in the bank), then merge
            into the SBUF accumulator (and normalize/store on the diagonal
            phase e == tp)."""
            php = ps.tile([128, 4 * 65], F32, tag="php", bufs=2,
                          name=f"php{rep}_{e}_{tp}")
            for qb in range(4):
                go = 2 * (4 * tp + qb) + 1
                mms = []
                for k in ks:
                    _, m, c0 = pairs[k]
                    for i, s in ((0, 2 * m), (1, 2 * m + 1)):
                        if s <= go:
                            mms.append((k, i, s, c0))
                for j, (k, i, s, c0) in enumerate(mms):
                    nc.tensor.matmul(
                        php[:, 65*qb:65*(qb+1)],
                        pts[k][:, 512*i+128*qb-c0:512*i+128*(qb+1)-c0],
                        pv_rhs(s),
                        start=(j == 0), stop=(j == len(mms) - 1))
            if e == 0:
                nc.vector.tensor_copy(accs[tp][:], php[:])
            else:
                nc.vector.scalar_tensor_tensor(accs[tp][:], php[:], 0.0,
                                               accs[tp][:], ALU.bypass, ALU.add)
            if e == tp:  # diagonal phase: normalize + store
                a3 = accs[tp][:].rearrange("p (q c) -> p q c", c=65)
                rc = sb.tile([128, 4], F32, tag="rc", bufs=2,
                             name=f"rc{rep}_{tp}")
                nc.vector.reciprocal(rc[:], a3[:, :, 64])
                for qb in range(4):
                    nc.vector.tensor_scalar_mul(ofins[tp][:, 64*qb:64*(qb+1)],
                                                a3[:, qb, 0:64], rc[:, qb:qb+1])
                if tp == NTT - 1:
                    nc.sync.dma_start(out_ap[128*tp:128*(tp+1), :], ofins[tp][:])
                else:
                    nc.gpsimd.dma_start(out_ap[128*tp:128*(tp+1), :], ofins[tp][:])

        # ACT (the sole exp engine) is the attention rate-limiter: keep its
        # queue pure exp and never input-starved; S runs two pairs ahead,
        # masks trail one pair, PV fires once a tile-phase's pairs are done.
        for k in range(min(2, n)):
            emit_S(k)
        for k in range(n + 1):
            if k < n:
                emit_exp_pair(k)
            if k >= 1:
                emit_mask(k - 1)
            if k + 2 < n:
                emit_S(k + 2)
            if k >= 1 and k % 4 == 0:
                kk = k - 4
                tp, m, _ = pairs[kk]
                emit_pv_phase(m // 4, tp, [kk, kk + 1, kk + 2, kk + 3])

    # projections first; SP-queue DMA order is explicit so xt loads beat
    # the bulk exchange traffic and the k half of exchange(0) goes earliest
    ck, cv = emit_proj(0)
    load_xt(1)
    ckv1 = emit_proj(1)
    emit_exchange0_k(ck)
    load_xt(2)
    ckv2 = emit_proj(2)
    emit_exchange0_v(cv)
    emit_exchange(1, ckv1)
    load_xt(3)
    ckv3 = emit_proj(3)
    emit_exchange(2, ckv2)
    emit_exchange(3, ckv3)
    if DEBUG_DUMP:
        kvd = nc.dram_tensor("kvdump", [128, 8 * 772], BF16,
                             kind="ExternalOutput").ap()
        qkd = nc.dram_tensor("qkdump", [128, 4 * 512], BF16,
                             kind="ExternalOutput").ap()
        nc.sync.dma_start(kvd[:], kv_sb[:])
        for t in range(NTT):
            nc.sync.dma_start(qkd[:, 512*t:512*(t+1)], qk_own[t][:])
    emit_attn_all()


DEBUG_DUMP = False


def build(reps=1, n_devices=N_CORES):
    nc = bacc.Bacc("TRN2", target_bir_lowering=False, debug=False,
                   num_devices=n_devices)
    xT_ap = nc.dram_tensor("xT", [C, NT], BF16, kind="ExternalInput").ap()
    wqk_ap = nc.dram_tensor("wqk", [128, 8 * 128], BF16,
                            kind="ExternalInput").ap()
    wv_ap = nc.dram_tensor("wv", [128, 8 * 64], BF16, kind="ExternalInput").ap()
    masks_ap = nc.dram_tensor("masks", [128, 2 * 128], BF16,
                              kind="ExternalInput").ap()
    # out rows: (tau, t) pairs; cols: (qb, h) -> local token = tau*512+qb*128+t
    out_ap = nc.dram_tensor("out", [NTT * 128, 4 * H], F32,
                            kind="ExternalOutput").ap()
    aps = (xT_ap, wqk_ap, wv_ap, masks_ap, out_ap)

    with tile.TileContext(nc) as tc:
        with tc.tile_pool(name="sb", bufs=1) as sb, \
             tc.tile_pool(name="ps", bufs=1, space="PSUM") as ps, \
             tc.tile_pool(name="dr", bufs=1, space="DRAM") as dr:
            for rep in range(reps):
                _emit_body(nc, tc, aps, (sb, ps, dr), rep)
    nc.compile()
    return nc


def make_inputs(x, Wq, Wk, Wv):
    """Per-core input maps from full inputs."""
    x = np.asarray(x, dtype=np.float32)
    Wq, Wk, Wv = (np.asarray(w, dtype=np.float32) for w in (Wq, Wk, Wv))
    # fold softmax scale and base-2 conversion into Wq: S' = log2(e)/sqrt(H)*qk
    wqk = np.concatenate([Wq * (LOG2E / np.sqrt(H)), Wk], axis=1)
    tril = (np.arange(128)[:, None] <= np.arange(128)[None, :]).astype(np.float32)
    zeros = np.zeros((128, 128), np.float32)
    ones = np.ones((128, 128), np.float32)
    masks_even = np.concatenate([tril, zeros], axis=1)   # p=0: diag at even s
    masks_odd = np.concatenate([ones, tril], axis=1)     # p=1: diag at odd s

    ml = mybir.dt.np(BF16)
    # pre-arrange weights into the SBUF chunk layout [128, chunks*cols]
    wqk16 = np.ascontiguousarray(
        wqk.reshape(8, 128, 128).transpose(1, 0, 2).reshape(128, 1024)).astype(ml)
    wv16 = np.ascontiguousarray(
        Wv.reshape(8, 128, 64).transpose(1, 0, 2).reshape(128, 512)).astype(ml)

    in_maps = []
    for core in range(N_CORES):
        b, p = core // 2, core % 2
        xT = np.ascontiguousarray(
            x[b].T.reshape(C, NBLK, 128)[:, p::2, :].reshape(C, NT)).astype(ml)
        in_maps.append({
            "xT": xT, "wqk": wqk16, "wv": wv16,
            "masks": (masks_even if p == 0 else masks_odd).astype(ml),
        })
    return in_maps


def gather_output(results):
    """results: list per core of {"out": [512, 256]} -> [B, T, H]."""
    out = np.empty((B, T, H), dtype=np.float32)
    for core in range(N_CORES):
        b, p = core // 2, core % 2
        o = results[core]["out"].reshape(NTT, 128, 4, H)
        o = o.transpose(0, 2, 1, 3).reshape(NLOC, 128, H)
        out[b].reshape(NBLK, 128, H)[p::2] = o
    return out


# ---------------------------------------------------------------------------
# held PJRT runner (axon path) — inlined so kernel.py is self-contained
# ---------------------------------------------------------------------------

def make_runner(nc, n_cores):
    import jax
    from jax.sharding import Mesh, PartitionSpec
    from jax.experimental.shard_map import shard_map
    from concourse import bass2jax
    from concourse.bass2jax import _bass_exec_p, install_neuronx_cc_hook

    install_neuronx_cc_hook()
    partition_name = nc.partition_id_tensor.name if nc.partition_id_tensor else None

    in_names, out_names, out_avals, zero_shapes = [], [], [], []
    for alloc in nc.m.functions[0].allocations:
        if not isinstance(alloc, mybir.MemoryLocationSet):
            continue
        name = alloc.memorylocations[0].name
        if alloc.kind == "ExternalInput":
            if name != partition_name:
                in_names.append(name)
        elif alloc.kind == "ExternalOutput":
            out_names.append(name)
            shape = tuple(alloc.tensor_shape)
            dtype = mybir.dt.np(alloc.dtype)
            out_avals.append(jax.core.ShapedArray(shape, dtype))
            zero_shapes.append((shape, dtype))
    n_params, n_outs = len(in_names), len(out_avals)
    all_in_names = list(in_names) + list(out_names)
    if partition_name is not None:
        all_in_names.append(partition_name)
    donate = tuple(range(n_params, n_params + n_outs))

    def _body(*args):
        operands = list(args)
        if partition_name is not None:
            operands.append(bass2jax.partition_id_tensor())
        outs = _bass_exec_p.bind(
            *operands, out_avals=tuple(out_avals), in_names=tuple(all_in_names),
            out_names=tuple(out_names), lowering_input_output_aliases=(),
            sim_require_finite=True, sim_require_nnan=True, nc=nc)
        return tuple(outs)

    devices = jax.devices()[:n_cores]
    mesh = Mesh(np.asarray(devices), ("core",))
    sharded = jax.jit(
        shard_map(_body, mesh=mesh,
                  in_specs=(PartitionSpec("core"),) * (n_params + n_outs),
                  out_specs=(PartitionSpec("core"),) * n_outs, check_rep=False),
        donate_argnums=donate, keep_unused=True)
    make_zeros = jax.jit(lambda: tuple(
        jax.numpy.zeros((n_cores * s[0], *s[1:]), d) for (s, d) in zero_shapes))

    class Runner:
        def commit_inputs(self, in_maps):
            per_core = [[np.asarray(m[name]) for name in in_names] for m in in_maps]
            concat = [np.concatenate([per_core[c][i] for c in range(n_cores)], axis=0)
                      for i in range(n_params)]
            self._committed = [jax.device_put(a) for a in concat]
            jax.block_until_ready(self._committed)

        def run(self):
            outs = sharded(*self._committed, *make_zeros())
            jax.block_until_ready(outs)
            return outs

        def results(self, outs):
            res = [dict() for _ in range(n_cores)]
            for i, name in enumerate(out_names):
                per = np.split(np.asarray(outs[i]), n_cores, axis=0)
                for c in range(n_cores):
                    res[c][name] = per[c]
            return res

    return Runner()


_cache = {}


def get_runner(reps=1):
    if reps not in _cache:
        nc = build(reps)
        _cache[reps] = make_runner(nc, N_CORES)
    return _cache[reps]


def kernel(x, Wq, Wk, Wv):
    r = get_runner(1)
    r.commit_inputs(make_inputs(x, Wq, Wk, Wv))
    return gather_output(r.results(r.run()))


# revision 54
# speedup vs baseline: 1.3903x; 1.0177x over previous
"""Causal single-head attention (B=4, T=4096, C=1024, H=64) on 8 TRN2 NeuronCores.

Sharding: core = 2*b + p handles batch b and the 16 query/key row-blocks
(128 rows each) of parity p (block-cyclic over T for causal load balance).
The instruction stream is parity-agnostic (SPMD); causality parity is
carried by per-core 0/1 mask data (p=0: [tril, zeros], p=1: [ones, tril]).

All activations/weights in bf16 (x converted on host). Per t-tile of 512
local tokens the core projects q^T,k^T (transposed, H-major) and v
(token-major) from one prefetched x^T slice, the core pair exchanges
k^T/v per tile via one AllGather (bf16), and attention runs per key
block s with near-exact causal spans:
    S^T[s,*] = kT_s.T @ qT[span]      (bf16 matmul, f32 psum)
    P^T = 2^(S^T)                     (scale 1/(8*ln2... log2e/8) folded
                                       into Wq; split ACT exp(ln2*x) and
                                       DVE pow(2,x) by greedy balance)
    mask[s%2] on first 128 cols       (DVE bf16 mul)
    out[q,65] += P^T_block.T @ [v|1]  (weights-stationary, 65-cycle matmuls;
                                       col 64 accumulates softmax denom)
then reciprocal-normalize (DVE) and one store per t-tile (Pool swdge).
"""
import numpy as np

import concourse.bacc as bacc
import concourse.bass as bass
import concourse.mybir as mybir
import concourse.tile as tile

dt = mybir.dt
BF16 = dt.bfloat16
F32 = dt.float32

B, T, C, H = 4, 4096, 1024, 64
NBLK = T // 128            # 32 global blocks per batch
NLOC = NBLK // 2           # 16 blocks per core
NT = NLOC * 128            # 2048 query rows per core
NTT = NT // 512            # 4 t-tiles per core
N_CORES = 8
GROUPS = [[0, 1], [2, 3], [4, 5], [6, 7]]
LOG2E = float(np.log2(np.e))
LN2 = float(np.log(2.0))

EXP = mybir.ActivationFunctionType.Exp
ALU = mybir.AluOpType

# exp engine balance: effective ns per psum column + per-op overhead,
# plus an initial credit for each engine's non-exp work. GPSIMD cannot
# read PSUM on hardware, so only ACT and DVE run the exp.
EXP_COST = [  # (ns_per_col, ns_per_op, initial_credit)
    (1 / 1.2, 185.0, 0.0),        # ACT
    (1 / 0.96, 125.0, 16000.0),   # DVE: copies + half the masks + normalize
]


def _pairs(tau):
    """Key blocks (2m, 2m+1) share one span: (m, col0 within the 512-tile).
    Parity-agnostic superset structure (p=1 exact; p=0 cores zero the s-odd
    diagonal via mask data)."""
    return [(m, 128 * max(0, m - 4 * tau)) for m in range(4 * tau + 4)]


def _emit_body(nc, tc, aps, pools, rep):
    (xT_ap, wqk_ap, wv_ap, masks_ap, out_ap) = aps
    sb, ps, dr = pools

    # --- constants ---
    wqk = sb.tile([128, 8 * 128], BF16, tag="wqk", name=f"wqk{rep}")
    wv = sb.tile([128, 8 * 64], BF16, tag="wv", name=f"wv{rep}")
    masks = sb.tile([128, 2 * 128], BF16, tag="masks", name=f"masks{rep}")
    two = sb.tile([128, 1], BF16, tag="two", name=f"two{rep}")
    nc.sync.dma_start(wqk[:], wqk_ap[:])
    nc.sync.dma_start(wv[:], wv_ap[:])
    nc.sync.dma_start(masks[:], masks_ap[:])
    nc.vector.memset(two[:], 2.0)

    # --- persistent activations ---
    # kv_sb: packed exchange mirror, one 772-col region per (tau, j):
    #   cols [0:260]   = [v|1] per key block q: [65q : 65q+65] (token-major)
    #   cols [260:772] = kT, rows 0:64 (H-major, 128 cols per block q);
    #   rows 64:128 of that range ride along unused (keeps matmul operands
    #   at base partition 0)
    kv_sb = sb.tile([128, 8 * 772], BF16, tag="kv", name=f"kv{rep}")

    def kv_base(s):
        return 772 * (2 * (s // 8) + (s % 2))

    def pv_rhs(s):
        q = (s % 8) // 2
        return kv_sb[:, kv_base(s)+65*q:kv_base(s)+65*q+65]

    def s_lhsT(s):
        q = (s % 8) // 2
        c = kv_base(s) + 260 + 128 * q
        return kv_sb[0:64, c:c+128]

    # x tiles: loaded in halves, staggered (xt0 upfront, xt(tau+1) after
    # proj(tau)'s exchange DMAs so the exchange wins the DMA-engine queue)
    xT_3d = xT_ap[:].rearrange("(g p) n -> p g n", p=128)          # [128,8,NT]
    xts = [sb.tile([128, 8 * 512], BF16, tag=f"xt{tau}", name=f"xt{rep}_{tau}")
           for tau in range(NTT)]

    def load_xt(tau):
        for h in range(4):
            nc.sync.dma_start(
                xts[tau][:].rearrange("p (g n) -> p g n", g=8)[:, 2*h:2*h+2, :],
                xT_3d[:, 2*h:2*h+2, 512*tau:512*tau+512])

    load_xt(0)

    qk_own = [sb.tile([128, 512], BF16, tag=f"qk{tau}", name=f"qk{rep}_{tau}")
              for tau in range(NTT)]

    # warm up the PE p-state ramp while weights/x stream in; scratch input
    # comes from a memset so the warmup has no DMA dependency
    wsrc = sb.tile([128, 512], BF16, tag="wsrc", name=f"wsrc{rep}")
    nc.vector.memset(wsrc[:], 0.25)
    warm = ps.tile([128, 1024], F32, tag="sc", bufs=2, name=f"warm{rep}")
    for c in range(8):
        nc.tensor.matmul(warm[:, 0:512], wsrc[:, 0:128], wsrc[:],
                         start=(c == 0), stop=(c == 7))

    # ---- stage A: projections + pair exchange for one t-tile ----
    # compute (PE matmuls + psum->sbuf copies + Pool stages) is emitted per
    # tau; the SP-queue exchange DMAs (stand-ins/collective readbacks) are
    # emitted separately so x-tile loads win the DMA-engine queue early on
    def emit_proj(tau):
        xt = xts[tau]
        qkp = ps.tile([128, 512], F32, tag="qkp", bufs=1, name=f"qkp{rep}_{tau}")
        vp = ps.tile([128, 256], F32, tag="vp", bufs=1,
                     name=f"vp{rep}_{tau}")
        for c in range(8):
            nc.tensor.matmul(qkp[:], wqk[:, 128*c:128*(c+1)], xt[:, 512*c:512*(c+1)],
                             start=(c == 0), stop=(c == 7))
        for tb in range(4):
            for c in range(8):
                nc.tensor.matmul(vp[:, 64*tb:64*(tb+1)],
                                 xt[:, 512*c+128*tb:512*c+128*(tb+1)],
                                 wv[:, 64*c:64*(c+1)],
                                 start=(c == 0), stop=(c == 7))
        # psum -> sbuf bf16; v_own carries the ones column per block
        v_own = sb.tile([128, 260], BF16, tag="vown", bufs=2, name=f"vo{rep}_{tau}")
        nc.vector.tensor_copy(qk_own[tau][:], qkp[:])
        v_own3 = v_own[:].rearrange("p (q c) -> p q c", c=65)
        nc.vector.tensor_copy(v_own3[:, :, 0:64],
                              vp[:].rearrange("p (q c) -> p q c", q=4))
        nc.vector.memset(v_own3[:, :, 64], 1.0)

        if tau == 0:
            # first exchange gates attention: k ships separately (S needs
            # only k), v follows while the first exps run
            ck = dr.tile([64, 512], BF16, tag="ck", name=f"ck{rep}")
            cv = dr.tile([128, 260], BF16, tag="cv", name=f"cv{rep}")
            nc.gpsimd.dma_start(ck[:], qk_own[0][64:128, :])
            nc.gpsimd.dma_start(cv[:], v_own[:])
            return (ck, cv)
        ckv = dr.tile([128, 772], BF16, tag="ckv", bufs=2, name=f"ckv{rep}_{tau}")
        nc.gpsimd.dma_start(ckv[:, 0:260], v_own[:])
        nc.gpsimd.dma_start(ckv[0:64, 260:772], qk_own[tau][64:128, :])
        return ckv

    def emit_exchange0_k(ck):
        ck_o = dr.tile([2, 64, 512], BF16, tag="cko", name=f"cko{rep}")
        if nc.num_devices > 1:
            nc.gpsimd.collective_compute(
                "AllGather", ALU.bypass, replica_groups=GROUPS,
                ins=[ck[:]], outs=[ck_o[:]])
        else:  # single-core timing sim: stand-ins, same traffic
            nc.sync.dma_start(ck_o[0], ck[:])
            nc.sync.dma_start(ck_o[1], ck[:])
        for j in (0, 1):
            nc.sync.dma_start(kv_sb[0:64, 772*j+260:772*j+772], ck_o[j])

    def emit_exchange0_v(cv):
        cv_o = dr.tile([2, 128, 260], BF16, tag="cvo", name=f"cvo{rep}")
        if nc.num_devices > 1:
            nc.gpsimd.collective_compute(
                "AllGather", ALU.bypass, replica_groups=GROUPS,
                ins=[cv[:]], outs=[cv_o[:]])
        else:
            nc.sync.dma_start(cv_o[0], cv[:])
            nc.sync.dma_start(cv_o[1], cv[:])
        for j in (0, 1):
            nc.sync.dma_start(kv_sb[:, 772*j:772*j+260], cv_o[j])

    def emit_exchange(tau, ckv):
        ccout = dr.tile([2, 128, 772], BF16, tag="ccout", bufs=2,
                        name=f"ccout{rep}_{tau}")
        if nc.num_devices > 1:
            nc.gpsimd.collective_compute(
                "AllGather", ALU.bypass, replica_groups=GROUPS,
                ins=[ckv[:]], outs=[ccout[:]],
            )
        else:  # single-core timing sim: stand-in DMAs, same traffic
            nc.sync.dma_start(ccout[0], ckv[:])
            nc.sync.dma_start(ccout[1], ckv[:])
        # both pair slices, incl. own (parity-agnostic), contiguous
        for j in (0, 1):
            base = 772 * (2 * tau + j)
            nc.sync.dma_start(kv_sb[:, base:base+772], ccout[j])

    # ---- stage B: attention, ordered by exchange phase ----
    # After exchange(e) lands, blocks s in [8e, 8e+8) are computable for every
    # tile tp >= e; exchange(e+1) hides behind that whole phase. Key blocks
    # are processed in (2m, 2m+1) pairs sharing one span so exp runs as one
    # ACT op per pair half. A psum bank holds only ONE open accumulation
    # group at a time (hardware: a start=True wipes the bank's open context),
    # so PV accumulates per (phase, tile) into a psum partial with strictly
    # sequential per-qb groups, then merges into SBUF running accumulators.
    def emit_attn_all(hooks):
        pairs = []                      # (tp, m, c0) in phase order
        for e in range(NTT):
            for tp in range(e, NTT):
                for m in range(4 * e, 4 * e + 4):
                    pairs.append((tp, m, 128 * max(0, m - 4 * tp)))
        n = len(pairs)
        accs = [sb.tile([128, 4 * 65], F32, tag=f"acc{tp}", name=f"acc{rep}_{tp}")
                for tp in range(NTT)]
        ofins = [sb.tile([128, 4 * 64], F32, tag=f"ofin{tp}", name=f"of{rep}_{tp}")
                 for tp in range(NTT)]
        sps, pts = [None] * n, [None] * n

        def emit_S(k):
            tp, m, c0 = pairs[k]
            w = 512 - c0
            sp_t = ps.tile([128, 1024], F32, tag="sc", bufs=2, name=f"sc{rep}_{k}")
            for i, s in ((0, 2 * m), (1, 2 * m + 1)):
                nc.tensor.matmul(sp_t[:, 512*i:512*i+w], s_lhsT(s),
                                 qk_own[tp][0:64, c0:512], start=True, stop=True)
            sps[k] = sp_t

        def emit_exp_pair(k):
            tp, m, c0 = pairs[k]
            w = 512 - c0
            pt_t = sb.tile([128, 1024], BF16, tag="pt", bufs=6, name=f"pt{rep}_{k}")
            sp3 = sps[k][:].rearrange("p (two c) -> p two c", two=2)[:, :, 0:w]
            pt3 = pt_t[:].rearrange("p (two c) -> p two c", two=2)[:, :, 0:w]
            nc.scalar.activation(pt3, sp3, EXP, scale=LN2)
            pts[k] = pt_t

        def emit_mask(k):
            tp, m, c0 = pairs[k]
            if m < 4 * tp:
                return  # off-diagonal pair: fully kept, no mask
            # both parities' diagonal query block (first 128 cols of each half)
            for i in (0, 1):
                nc.vector.tensor_mul(pts[k][:, 512*i:512*i+128],
                                     pts[k][:, 512*i:512*i+128],
                                     masks[:, 128*i:128*(i+1)])

        def emit_pv_phase(e, tp, ks):
            """PV for tile tp over this phase's four pairs `ks`, one complete
            psum group per qb (never two open groups in the bank), then merge
            into the SBUF accumulator (and normalize/store on the diagonal
            phase e == tp)."""
            php = ps.tile([128, 4 * 65], F32, tag="php", bufs=2,
                          name=f"php{rep}_{e}_{tp}")
            for qb in range(4):
                go = 2 * (4 * tp + qb) + 1
                mms = []
                for k in ks:
                    _, m, c0 = pairs[k]
                    for i, s in ((0, 2 * m), (1, 2 * m + 1)):
                        if s <= go:
                            mms.append((k, i, s, c0))
                for j, (k, i, s, c0) in enumerate(mms):
                    nc.tensor.matmul(
                        php[:, 65*qb:65*(qb+1)],
                        pts[k][:, 512*i+128*qb-c0:512*i+128*(qb+1)-c0],
                        pv_rhs(s),
                        start=(j == 0), stop=(j == len(mms) - 1))
            if e == 0:
                nc.vector.tensor_copy(accs[tp][:], php[:])
            else:
                nc.vector.scalar_tensor_tensor(accs[tp][:], php[:], 0.0,
                                               accs[tp][:], ALU.bypass, ALU.add)
            if e == tp:  # diagonal phase: normalize + store
                a3 = accs[tp][:].rearrange("p (q c) -> p q c", c=65)
                rc = sb.tile([128, 4], F32, tag="rc", bufs=2,
                             name=f"rc{rep}_{tp}")
                nc.vector.reciprocal(rc[:], a3[:, :, 64])
                for qb in range(4):
                    nc.vector.tensor_scalar_mul(ofins[tp][:, 64*qb:64*(qb+1)],
                                                a3[:, qb, 0:64], rc[:, qb:qb+1])
                if tp == NTT - 1:
                    nc.sync.dma_start(out_ap[128*tp:128*(tp+1), :], ofins[tp][:])
                else:
                    nc.gpsimd.dma_start(out_ap[128*tp:128*(tp+1), :], ofins[tp][:])

        # ACT (the sole exp engine) is the attention rate-limiter: keep its
        # queue pure exp and never input-starved; S runs two pairs ahead,
        # masks trail one pair, PV fires once a tile-phase's pairs are done.
        # Projections for tiles 1-3 and the remaining exchange DMAs are
        # injected into the stream (hooks) so exp starts right after proj0
        # + the k half of exchange(0).
        for k in range(min(2, n)):
            emit_S(k)
        for k in range(n + 1):
            if k in hooks:
                hooks[k]()
            if k < n:
                emit_exp_pair(k)
            if k >= 1:
                emit_mask(k - 1)
            if k + 2 < n:
                emit_S(k + 2)
            if k >= 1 and k % 4 == 0:
                kk = k - 4
                tp, m, _ = pairs[kk]
                emit_pv_phase(m // 4, tp, [kk, kk + 1, kk + 2, kk + 3])

    # proj0 + the k half of exchange(0) go first so the exp stream starts
    # as early as possible; later projections/exchanges are injected into
    # the attention stream right before their tile's first S matmuls
    ck, cv = emit_proj(0)
    load_xt(1)
    emit_exchange0_k(ck)
    st = {}

    def hook1():
        st["ckv1"] = emit_proj(1)
        load_xt(2)
        emit_exchange0_v(cv)

    def hook2():
        st["ckv2"] = emit_proj(2)
        load_xt(3)
        emit_exchange(1, st["ckv1"])

    def hook3():
        st["ckv3"] = emit_proj(3)
        emit_exchange(2, st["ckv2"])

    def hook4():
        emit_exchange(3, st["ckv3"])

    hooks = {2: hook1, 6: hook2, 10: hook3, 14: hook4}
    if DEBUG_DUMP:
        kvd = nc.dram_tensor("kvdump", [128, 8 * 772], BF16,
                             kind="ExternalOutput").ap()
        qkd = nc.dram_tensor("qkdump", [128, 4 * 512], BF16,
                             kind="ExternalOutput").ap()
        nc.sync.dma_start(kvd[:], kv_sb[:])
        for t in range(NTT):
            nc.sync.dma_start(qkd[:, 512*t:512*(t+1)], qk_own[t][:])
    emit_attn_all(hooks)


DEBUG_DUMP = False


def build(reps=1, n_devices=N_CORES):
    nc = bacc.Bacc("TRN2", target_bir_lowering=False, debug=False,
                   num_devices=n_devices)
    xT_ap = nc.dram_tensor("xT", [C, NT], BF16, kind="ExternalInput").ap()
    wqk_ap = nc.dram_tensor("wqk", [128, 8 * 128], BF16,
                            kind="ExternalInput").ap()
    wv_ap = nc.dram_tensor("wv", [128, 8 * 64], BF16, kind="ExternalInput").ap()
    masks_ap = nc.dram_tensor("masks", [128, 2 * 128], BF16,
                              kind="ExternalInput").ap()
    # out rows: (tau, t) pairs; cols: (qb, h) -> local token = tau*512+qb*128+t
    out_ap = nc.dram_tensor("out", [NTT * 128, 4 * H], F32,
                            kind="ExternalOutput").ap()
    aps = (xT_ap, wqk_ap, wv_ap, masks_ap, out_ap)

    with tile.TileContext(nc) as tc:
        with tc.tile_pool(name="sb", bufs=1) as sb, \
             tc.tile_pool(name="ps", bufs=1, space="PSUM") as ps, \
             tc.tile_pool(name="dr", bufs=1, space="DRAM") as dr:
            for rep in range(reps):
                _emit_body(nc, tc, aps, (sb, ps, dr), rep)
    nc.compile()
    return nc


def make_inputs(x, Wq, Wk, Wv):
    """Per-core input maps from full inputs."""
    x = np.asarray(x, dtype=np.float32)
    Wq, Wk, Wv = (np.asarray(w, dtype=np.float32) for w in (Wq, Wk, Wv))
    # fold softmax scale and base-2 conversion into Wq: S' = log2(e)/sqrt(H)*qk
    wqk = np.concatenate([Wq * (LOG2E / np.sqrt(H)), Wk], axis=1)
    tril = (np.arange(128)[:, None] <= np.arange(128)[None, :]).astype(np.float32)
    zeros = np.zeros((128, 128), np.float32)
    ones = np.ones((128, 128), np.float32)
    masks_even = np.concatenate([tril, zeros], axis=1)   # p=0: diag at even s
    masks_odd = np.concatenate([ones, tril], axis=1)     # p=1: diag at odd s

    ml = mybir.dt.np(BF16)
    # pre-arrange weights into the SBUF chunk layout [128, chunks*cols]
    wqk16 = np.ascontiguousarray(
        wqk.reshape(8, 128, 128).transpose(1, 0, 2).reshape(128, 1024)).astype(ml)
    wv16 = np.ascontiguousarray(
        Wv.reshape(8, 128, 64).transpose(1, 0, 2).reshape(128, 512)).astype(ml)

    in_maps = []
    for core in range(N_CORES):
        b, p = core // 2, core % 2
        xT = np.ascontiguousarray(
            x[b].T.reshape(C, NBLK, 128)[:, p::2, :].reshape(C, NT)).astype(ml)
        in_maps.append({
            "xT": xT, "wqk": wqk16, "wv": wv16,
            "masks": (masks_even if p == 0 else masks_odd).astype(ml),
        })
    return in_maps


def gather_output(results):
    """results: list per core of {"out": [512, 256]} -> [B, T, H]."""
    out = np.empty((B, T, H), dtype=np.float32)
    for core in range(N_CORES):
        b, p = core // 2, core % 2
        o = results[core]["out"].reshape(NTT, 128, 4, H)
        o = o.transpose(0, 2, 1, 3).reshape(NLOC, 128, H)
        out[b].reshape(NBLK, 128, H)[p::2] = o
    return out


# ---------------------------------------------------------------------------
# held PJRT runner (axon path) — inlined so kernel.py is self-contained
# ---------------------------------------------------------------------------

def make_runner(nc, n_cores):
    import jax
    from jax.sharding import Mesh, PartitionSpec
    from jax.experimental.shard_map import shard_map
    from concourse import bass2jax
    from concourse.bass2jax import _bass_exec_p, install_neuronx_cc_hook

    install_neuronx_cc_hook()
    partition_name = nc.partition_id_tensor.name if nc.partition_id_tensor else None

    in_names, out_names, out_avals, zero_shapes = [], [], [], []
    for alloc in nc.m.functions[0].allocations:
        if not isinstance(alloc, mybir.MemoryLocationSet):
            continue
        name = alloc.memorylocations[0].name
        if alloc.kind == "ExternalInput":
            if name != partition_name:
                in_names.append(name)
        elif alloc.kind == "ExternalOutput":
            out_names.append(name)
            shape = tuple(alloc.tensor_shape)
            dtype = mybir.dt.np(alloc.dtype)
            out_avals.append(jax.core.ShapedArray(shape, dtype))
            zero_shapes.append((shape, dtype))
    n_params, n_outs = len(in_names), len(out_avals)
    all_in_names = list(in_names) + list(out_names)
    if partition_name is not None:
        all_in_names.append(partition_name)
    donate = tuple(range(n_params, n_params + n_outs))

    def _body(*args):
        operands = list(args)
        if partition_name is not None:
            operands.append(bass2jax.partition_id_tensor())
        outs = _bass_exec_p.bind(
            *operands, out_avals=tuple(out_avals), in_names=tuple(all_in_names),
            out_names=tuple(out_names), lowering_input_output_aliases=(),
            sim_require_finite=True, sim_require_nnan=True, nc=nc)
        return tuple(outs)

    devices = jax.devices()[:n_cores]
    mesh = Mesh(np.asarray(devices), ("core",))
    sharded = jax.jit(
        shard_map(_body, mesh=mesh,
                  in_specs=(PartitionSpec("core"),) * (n_params + n_outs),
                  out_specs=(PartitionSpec("core"),) * n_outs, check_rep=False),
        donate_argnums=donate, keep_unused=True)
    make_zeros = jax.jit(lambda: tuple(
        jax.numpy.zeros((n_cores * s[0], *s[1:]), d) for (s, d) in zero_shapes))

    class Runner:
        def commit_inputs(self, in_maps):
            per_core = [[np.asarray(m[name]) for name in in_names] for m in in_maps]
            concat = [np.concatenate([per_core[c][i] for c in range(n_cores)], axis=0)
                      for i in range(n_params)]
            self._committed = [jax.device_put(a) for a in concat]
            jax.block_until_ready(self._committed)

        def run(self):
            outs = sharded(*self._committed, *make_zeros())
            jax.block_until_ready(outs)
            return outs

        def results(self, outs):
            res = [dict() for _ in range(n_cores)]
            for i, name in enumerate(out_names):
                per = np.split(np.asarray(outs[i]), n_cores, axis=0)
                for c in range(n_cores):
                    res[c][name] = per[c]
            return res

    return Runner()


_cache = {}


def get_runner(reps=1):
    if reps not in _cache:
        nc = build(reps)
        _cache[reps] = make_runner(nc, N_CORES)
    return _cache[reps]


def kernel(x, Wq, Wk, Wv):
    r = get_runner(1)
    r.commit_inputs(make_inputs(x, Wq, Wk, Wv))
    return gather_output(r.results(r.run()))
